# revision 22
# baseline (speedup 1.0000x reference)
"""Trainium2 Bass kernel: weighted BCE + IoU loss (structure loss).

Full inputs: pred/mask [64, 1, 512, 512] fp32.  Data-parallel over 8
NeuronCores (8 images per core).

Host-side prep (like the baseline's lmp/pm/rm1 channels) ships two
bf16 channels per image:
  M  = mask
  TB = tb = sp - P*M - gu        (pointwise; sp = log1p(e^P),
       gu = (e^P*(M+1)+1)/(2e^P+M+1);  bce+iou = tb + 1)

Device per image (the structural work):
  - H-direction 31-tap box filter: banded matmul on PE (f32 PSUM),
    evacuated with a 1/961 scale into a zero-gapped SBUF strip (ACT).
  - W-direction box filter: ONE tensor_tensor_scan with data1 a
    31-shifted view of the strip (running 31-window sum; zero gaps
    make edge handling automatic).
  - d  = sc - M                   (TensorTensor sub, 2x mode)
  - ad = |d|, acc A = sum(ad)     (TensorScalar abs_max, 4x mode)
  - W  = sum((ad + 0.2) * TB)     (scalar_tensor_tensor w/ accum)
  - final 128-partition reduce of [A_i; W_i] via ones-matmul on PE.

Host finish:  loss_i = 5*(W + A + 0.2*HW) / (HW + 5*A),  output mean.
"""

import os as _os
from contextlib import ExitStack

import numpy as np

_B = 64
_H = 512
_W = 512
_NC = 8
_BPC = _B // _NC
_HW = float(_H * _W)

_W1 = 2224   # gapped strip width
_L = 2192    # sc tile width
_LS = 2156   # scan op length
_GAP = 31
_STRIDE = 543  # 512 + 31

_CACHE = {}

# knobs (engine placement per image index)
def _imgset(env, default):
    return {int(x) for x in _os.environ.get(env, default).split(",") if x != ""}

_D_DVE = _imgset("K_D_DVE", "")            # d-sub on DVE instead of Pool
_ABSD_IMGS = _imgset("K_ABSD", "")         # ABSD custom (DVE) instead of d+ACT-Abs
_EVAC = _os.environ.get("K_EVAC", "a,a,a,a,a,a,a,a")
_EVAC_MAP = (_EVAC.split(",") * 8)[:8] if "," in _EVAC else [_EVAC[0]] * 8
_IBUFS = int(_os.environ.get("K_IBUFS", "8"))
_VBUFS = int(_os.environ.get("K_VBUFS", "4"))
_PBUFS = int(_os.environ.get("K_PBUFS", "3"))
_SBUFS = int(_os.environ.get("K_SBUFS", "3"))
_DBUFS = int(_os.environ.get("K_DBUFS", "3"))
_SKEW = int(_os.environ.get("K_SKEW", "1"))
_WARM = _os.environ.get("K_WARM", "1") == "1"
_PEWARM = int(_os.environ.get("K_PEWARM", "12"))


def _band_np():
    import ml_dtypes

    idx = np.arange(_H)
    b = (np.abs(idx[:, None] - idx[None, :]) <= 15).astype(np.float32)
    return b.astype(ml_dtypes.bfloat16)


def _register_custom_ops():
    """Register the ABSD custom DVE op (idempotent):
    ap2 = |Src0*C0 - Src1| + C1, accum_out = sum(ap2)."""
    import concourse.dve_ops as dops
    from concourse.dve_spec import AluOp, Spec, Src0, Src1, Zero, lower, maxx
    from concourse.dve_uop import DveOpSpec
    from operator import add as _add

    if "ABSD_ANT" in dops.CUSTOM_DVE_SPECS:
        return

    from concourse.dve_spec import C0, C1

    dd = Src0 * C0 - Src1

    def _ref_absd(in0, in1, c0, c1, c2):
        b = (np.abs(in0.astype(np.float32) * c0 - in1) + c1).astype(np.float32)
        return b, b.reshape(b.shape[0], -1).sum(axis=-1, keepdims=True)

    absd = dops.DveOp(
        "ABSD_ANT",
        Spec(body=maxx(dd, Zero - dd) + C1, accum=_add, accum_init=Zero,
             reference=_ref_absd),
        subdim=False, uops_sha={},
    )

    uops = lower(absd.spec, ver="v3")
    sha = DveOpSpec(name=absd.name, opcode=0, uops=uops, rd1_en=True).sha("v3")
    pinned = dops.DveOp(absd.name, absd.spec, absd.subdim, {"v3": sha})

    base = max(dops._SUB_OPCODE_FOR_NAME.values())
    dops.OPS.append(pinned)
    dops.CUSTOM_DVE_SPECS[pinned.name] = pinned.spec
    dops._SUB_OPCODE_FOR_NAME[pinned.name] = base + 1
    assert max(dops._SUB_OPCODE_FOR_NAME.values()) < 0x20


def _build():
    if "nc" in _CACHE:
        return _CACHE["nc"]

    import concourse.bass as bass
    import concourse.tile as tile
    from concourse import bacc, mybir
    from concourse.alu_op_type import AluOpType as ALU

    AF = mybir.ActivationFunctionType
    MALU = mybir.AluOpType
    F32 = mybir.dt.float32
    BF16 = mybir.dt.bfloat16
    ts = bass.ts

    import concourse.dve_ops as dops

    _register_custom_ops()
    ABSD = next(o for o in dops.OPS if o.name == "ABSD_ANT")

    nc = bacc.Bacc("TRN2", target_bir_lowering=False, debug=False,
                   num_devices=_NC)

    mask_d = nc.dram_tensor("mask", [_BPC, _H, _W], BF16, kind="ExternalInput").ap()
    tb_d = nc.dram_tensor("tb", [_BPC, _H, _W], BF16, kind="ExternalInput").ap()
    band_d = nc.dram_tensor("band", [_H, _W], BF16, kind="ExternalInput").ap()
    out_d = nc.dram_tensor("out", [1, 2 * _BPC], F32, kind="ExternalOutput").ap()

    with tile.TileContext(nc) as tc, ExitStack() as ctx:
        cpool = ctx.enter_context(tc.tile_pool(name="cpool", bufs=1))
        ipool = ctx.enter_context(tc.tile_pool(name="ipool", bufs=_IBUFS))
        vpool = ctx.enter_context(tc.tile_pool(name="vpool", bufs=_VBUFS))
        spool = ctx.enter_context(tc.tile_pool(name="spool", bufs=_SBUFS))
        dpool = ctx.enter_context(tc.tile_pool(name="dpool", bufs=_DBUFS))
        apool = ctx.enter_context(tc.tile_pool(name="apool", bufs=_DBUFS))
        wpool = ctx.enter_context(tc.tile_pool(name="wpool", bufs=_DBUFS))
        pup = ctx.enter_context(tc.tile_pool(name="pup", bufs=_PBUFS, space="PSUM"))

        band_sb = cpool.tile([128, 4, _W], BF16, name="band_sb", tag="band_sb")
        nc.sync.dma_start(band_sb[:], band_d.rearrange("(j p) c -> p j c", p=128))
        acc = cpool.tile([128, 2 * _BPC], F32, name="acc", tag="acc")
        nc.gpsimd.memset(acc[:], 0.0)

        if _WARM:
            # touch Abs at t~0 so the ACT func table loads during the
            # initial DMA wait instead of on the first image's chain
            warm = cpool.tile([128, 1], BF16, name="warm", tag="warm")
            nc.gpsimd.memset(warm[:], 0.0)
            nc.scalar.activation(warm[:], warm[:], AF.Abs)

        if _PEWARM:
            # dummy matmuls ramp the PE pstate during the initial DMA wait
            # so image 0's real matmuls run at full speed
            wsrc = cpool.tile([128, _W], BF16, name="wsrc", tag="wsrc")
            nc.gpsimd.memset(wsrc[:], 0.0)
            wps = pup.tile([128, 2, _W], F32, name="wps", tag="v1ps")
            for n in range(_PEWARM):
                nc.tensor.matmul(out=wps[:, n % 2, :], lhsT=wsrc[:, 0:128],
                                 rhs=wsrc[:], start=True, stop=True)

        # pre-zero only the strip GAPS (interiors are overwritten each
        # image): head [0:31], the three inter-row gaps (strided view),
        # and the tail — per rotating buffer
        for b in range(_VBUFS):
            v1p0 = vpool.tile([128, _W1], BF16, name=f"v1p{b}", tag="v1p")
            nc.gpsimd.memset(v1p0[:, 0:_GAP], 0.0)
            gaps = v1p0[:, _STRIDE : _STRIDE + 3 * _STRIDE].rearrange(
                "p (j s) -> p j s", s=_STRIDE)[:, :, 0:_GAP]
            nc.gpsimd.memset(gaps, 0.0)
            nc.gpsimd.memset(v1p0[:, _GAP + 3 * _STRIDE + _W : _W1], 0.0)

        def front(i):
            """loads + H-filter matmuls + evac + W-scan for image i."""
            mb = ipool.tile([128, 4, _W], BF16, name="mb", tag="mb")
            nc.sync.dma_start(mb[:], mask_d[i].rearrange("(j p) w -> p j w", p=128))
            tbb = ipool.tile([128, 4, _W], BF16, name="tbb", tag="tbb")
            nc.sync.dma_start(tbb[:], tb_d[i].rearrange("(j p) w -> p j w", p=128))

            v1p = vpool.tile([128, _W1], BF16, name="v1p", tag="v1p")
            for k in range(2):  # row pairs (ih = 2k, 2k+1)
                v1ps = pup.tile([128, 2, _W], F32, name="v1ps", tag="v1ps")
                for ii in range(2):
                    ih = 2 * k + ii
                    js = [j for j in (ih - 1, ih, ih + 1) if 0 <= j < 4]
                    for n, j in enumerate(js):
                        nc.tensor.matmul(
                            out=v1ps[:, ii, :],
                            lhsT=band_sb[:, j, ts(ih, 128)],
                            rhs=mb[:, j, :],
                            start=(n == 0),
                            stop=(n == len(js) - 1),
                        )
                interior = v1p[
                    :, _GAP + 2 * k * _STRIDE : _GAP + (2 * k + 2) * _STRIDE
                ].rearrange("p (j w) -> p j w", w=_STRIDE)[:, :, 0:_W]
                if _EVAC_MAP[i] == "a":
                    nc.scalar.activation(interior, v1ps[:], AF.Copy,
                                         scale=1.0 / 961.0)
                else:
                    nc.vector.tensor_scalar(
                        out=interior, in0=v1ps[:], scalar1=1.0 / 961.0,
                        scalar2=None, op0=MALU.mult)

            sc = spool.tile([128, _L], BF16, name="sc", tag="sc")
            nc.vector.tensor_tensor_scan(
                out=sc[:, 0:_LS],
                data0=v1p[:, _GAP : _GAP + _LS],
                data1=v1p[:, 0:_LS],
                initial=0.0,
                op0=MALU.add,
                op1=MALU.subtract,
            )
            scv = sc[:, 15 : 15 + 4 * _STRIDE].rearrange(
                "p (j w) -> p j w", w=_STRIDE
            )[:, :, 0:_W]
            return mb, tbb, scv

        def back(i, mb, tbb, scv):
            """|sc - M| (+0.2) and the weighted sum for image i."""
            ad = apool.tile([128, 4, _W], BF16, name="ad", tag="ad")
            if i in _ABSD_IMGS:
                # fused d + abs + 0.2 + accum on DVE (1x custom);
                # acc[2i] = sum(|d| + 0.2)
                nc.vector._custom_dve(
                    ABSD, out=ad[:], in0=scv, in1=mb[:], s0=1.0, s1=0.2,
                    accum_out=acc[:, 2 * i : 2 * i + 1])
                w_bias = 0.0
            else:
                # d on Pool (TT sub), |d| + accum on ACT; acc[2i] = sum |d|
                d = dpool.tile([128, 4, _W], BF16, name="d", tag="d")
                deng = nc.vector if i in _D_DVE else nc.gpsimd
                deng.tensor_tensor(out=d[:], in0=scv, in1=mb[:],
                                   op=MALU.subtract)
                nc.scalar.activation(ad[:], d[:], AF.Abs,
                                     accum_out=acc[:, 2 * i : 2 * i + 1])
                w_bias = 0.2

            # w = (ad + w_bias) * tb, acc[2i+1] = sum(w)   (DVE custom)
            wout = wpool.tile([128, 4, _W], BF16, name="wout", tag="wout")
            nc.vector.affine_mul_reduce(
                out=wout[:], accum_out=acc[:, 2 * i + 1 : 2 * i + 2],
                in0=ad[:], in1=tbb[:], scale=1.0, bias=w_bias)

        # software pipeline: back(i) is emitted _SKEW images behind front
        # so no engine queue head-of-line blocks the next image's front
        inflight = {}
        for i in range(_BPC + _SKEW):
            if i < _BPC:
                inflight[i] = front(i)
            j = i - _SKEW
            if j >= 0:
                back(j, *inflight.pop(j))

        # -------- final 128-partition reduction (Pool, no PSUM) --------
        res = cpool.tile([1, 2 * _BPC], F32, name="res", tag="res")
        nc.gpsimd.tensor_reduce(out=res[:], in_=acc[:],
                                axis=mybir.AxisListType.C, op=MALU.add)
        nc.sync.dma_start(out_d[:], res[:])

    nc.compile()
    _CACHE["nc"] = nc
    return nc


def _prep_inputs(pred, mask):
    import ml_dtypes

    bf16 = ml_dtypes.bfloat16
    p = np.asarray(pred, np.float32).reshape(_B, _H, _W)
    m = np.asarray(mask, np.float32).reshape(_B, _H, _W)
    mb = np.ascontiguousarray(m.astype(bf16))
    # tb = sp - P*M - gu  (fp32 host math, one bf16 rounding at the end)
    E = np.exp(p)
    sp = np.log1p(E)
    gu = (E * (m + 1.0) + 1.0) / (2.0 * E + m + 1.0)
    tb = np.ascontiguousarray((sp - p * m - gu).astype(bf16))
    return mb, tb


def run_cores(pred, mask, trace=False, tmpdir=None):
    from concourse.bass_utils import run_bass_kernel_spmd

    nc = _build()
    mb, tb = _prep_inputs(pred, mask)
    band = _band_np()
    sl = lambda a, c: a[c * _BPC : (c + 1) * _BPC]
    in_maps = [
        {"mask": sl(mb, c), "tb": sl(tb, c), "band": band}
        for c in range(_NC)
    ]
    kw = {}
    if trace:
        kw = dict(trace=True, trace_cores=[0], tmpdir=tmpdir)
    br = run_bass_kernel_spmd(nc, in_maps, list(range(_NC)), **kw)
    outs = [br.results[c]["out"].reshape(2 * _BPC) for c in range(_NC)]
    return outs, br


def finish(outs):
    losses = []
    for c in range(_NC):
        o = outs[c].astype(np.float64)
        for i in range(_BPC):
            W = o[2 * i + 1]      # sum (|.| + 0.2) * tb
            if i in _ABSD_IMGS:
                sa = o[2 * i] - 0.2 * _HW   # acc held sum(|d| + 0.2)
            else:
                sa = o[2 * i]               # acc held sum |d|
            losses.append(5.0 * (W + sa + 0.2 * _HW) / (_HW + 5.0 * sa))
    return np.float32(np.mean(losses))


def kernel(pred, mask):
    outs, _ = run_cores(pred, mask)
    return finish(outs)


# revision 26
# speedup vs baseline: 1.0559x; 1.0559x over previous
"""Trainium2 Bass kernel: weighted BCE + IoU loss (structure loss).

Full inputs: pred/mask [64, 1, 512, 512] fp32.  Data-parallel over 8
NeuronCores (8 images per core).

Host-side prep (like the baseline's lmp/pm/rm1 channels) ships two
bf16 channels per image:
  M  = mask
  TB = tb = sp - P*M - gu        (pointwise; sp = log1p(e^P),
       gu = (e^P*(M+1)+1)/(2e^P+M+1);  bce+iou = tb + 1)

Device per image (the structural work):
  - H-direction 31-tap box filter: banded matmul on PE (f32 PSUM),
    evacuated with a 1/961 scale into a zero-gapped SBUF strip (ACT).
  - W-direction box filter: ONE tensor_tensor_scan with data1 a
    31-shifted view of the strip (running 31-window sum; zero gaps
    make edge handling automatic).
  - d  = sc - M                   (TensorTensor sub, 2x mode)
  - ad = |d|, acc A = sum(ad)     (TensorScalar abs_max, 4x mode)
  - W  = sum((ad + 0.2) * TB)     (scalar_tensor_tensor w/ accum)
  - final 128-partition reduce of [A_i; W_i] via ones-matmul on PE.

Host finish:  loss_i = 5*(W + A + 0.2*HW) / (HW + 5*A),  output mean.
"""

import os as _os
from contextlib import ExitStack

import numpy as np

_B = 64
_H = 512
_W = 512
_NC = 8
_BPC = _B // _NC
_HW = float(_H * _W)

_W1 = 2224   # gapped strip width
_L = 2192    # sc tile width
_LS = 2156   # scan op length
_GAP = 31
_STRIDE = 543  # 512 + 31

_CACHE = {}

# knobs (engine placement per image index)
def _imgset(env, default):
    return {int(x) for x in _os.environ.get(env, default).split(",") if x != ""}

_D_DVE = _imgset("K_D_DVE", "")            # d-sub on DVE instead of Pool
_ABSD_IMGS = _imgset("K_ABSD", "7")         # ABSD custom (DVE) instead of d+ACT-Abs
_EVAC = _os.environ.get("K_EVAC", "a,a,a,a,a,a,a,a")
_EVAC_MAP = (_EVAC.split(",") * 8)[:8] if "," in _EVAC else [_EVAC[0]] * 8
_IBUFS = int(_os.environ.get("K_IBUFS", "8"))
_VBUFS = int(_os.environ.get("K_VBUFS", "4"))
_PBUFS = int(_os.environ.get("K_PBUFS", "3"))
_SBUFS = int(_os.environ.get("K_SBUFS", "3"))
_DBUFS = int(_os.environ.get("K_DBUFS", "3"))
_SKEW = int(_os.environ.get("K_SKEW", "1"))
_WARM = _os.environ.get("K_WARM", "1") == "1"
_PEWARM = int(_os.environ.get("K_PEWARM", "0"))


def _band_np():
    import ml_dtypes

    idx = np.arange(_H)
    b = (np.abs(idx[:, None] - idx[None, :]) <= 15).astype(np.float32)
    return b.astype(ml_dtypes.bfloat16)


def _register_custom_ops():
    """Register the ABSD custom DVE op (idempotent):
    ap2 = |Src0*C0 - Src1| + C1, accum_out = sum(ap2)."""
    import concourse.dve_ops as dops
    from concourse.dve_spec import AluOp, Spec, Src0, Src1, Zero, lower, maxx
    from concourse.dve_uop import DveOpSpec
    from operator import add as _add

    if "ABSD_ANT" in dops.CUSTOM_DVE_SPECS:
        return

    from concourse.dve_spec import C0, C1

    dd = Src0 * C0 - Src1

    def _ref_absd(in0, in1, c0, c1, c2):
        b = (np.abs(in0.astype(np.float32) * c0 - in1) + c1).astype(np.float32)
        return b, b.reshape(b.shape[0], -1).sum(axis=-1, keepdims=True)

    absd = dops.DveOp(
        "ABSD_ANT",
        Spec(body=maxx(dd, Zero - dd) + C1, accum=_add, accum_init=Zero,
             reference=_ref_absd),
        subdim=False, uops_sha={},
    )

    uops = lower(absd.spec, ver="v3")
    sha = DveOpSpec(name=absd.name, opcode=0, uops=uops, rd1_en=True).sha("v3")
    pinned = dops.DveOp(absd.name, absd.spec, absd.subdim, {"v3": sha})

    base = max(dops._SUB_OPCODE_FOR_NAME.values())
    dops.OPS.append(pinned)
    dops.CUSTOM_DVE_SPECS[pinned.name] = pinned.spec
    dops._SUB_OPCODE_FOR_NAME[pinned.name] = base + 1
    assert max(dops._SUB_OPCODE_FOR_NAME.values()) < 0x20


def _build():
    if "nc" in _CACHE:
        return _CACHE["nc"]

    import concourse.bass as bass
    import concourse.tile as tile
    from concourse import bacc, mybir
    from concourse.alu_op_type import AluOpType as ALU

    AF = mybir.ActivationFunctionType
    MALU = mybir.AluOpType
    F32 = mybir.dt.float32
    BF16 = mybir.dt.bfloat16
    ts = bass.ts

    import concourse.dve_ops as dops

    _register_custom_ops()
    ABSD = next(o for o in dops.OPS if o.name == "ABSD_ANT")

    nc = bacc.Bacc("TRN2", target_bir_lowering=False, debug=False,
                   num_devices=_NC)

    mask_d = nc.dram_tensor("mask", [_BPC, _H, _W], BF16, kind="ExternalInput").ap()
    tb_d = nc.dram_tensor("tb", [_BPC, _H, _W], BF16, kind="ExternalInput").ap()
    band_d = nc.dram_tensor("band", [_H, _W], BF16, kind="ExternalInput").ap()
    out_d = nc.dram_tensor("out", [1, 2 * _BPC], F32, kind="ExternalOutput").ap()

    with tile.TileContext(nc) as tc, ExitStack() as ctx:
        cpool = ctx.enter_context(tc.tile_pool(name="cpool", bufs=1))
        ipool = ctx.enter_context(tc.tile_pool(name="ipool", bufs=_IBUFS))
        vpool = ctx.enter_context(tc.tile_pool(name="vpool", bufs=_VBUFS))
        spool = ctx.enter_context(tc.tile_pool(name="spool", bufs=_SBUFS))
        dpool = ctx.enter_context(tc.tile_pool(name="dpool", bufs=_DBUFS))
        apool = ctx.enter_context(tc.tile_pool(name="apool", bufs=_DBUFS))
        wpool = ctx.enter_context(tc.tile_pool(name="wpool", bufs=_DBUFS))
        pup = ctx.enter_context(tc.tile_pool(name="pup", bufs=_PBUFS, space="PSUM"))

        band_sb = cpool.tile([128, 4, _W], BF16, name="band_sb", tag="band_sb")
        nc.sync.dma_start(band_sb[:], band_d.rearrange("(j p) c -> p j c", p=128))
        acc = cpool.tile([128, 2 * _BPC], F32, name="acc", tag="acc")
        nc.gpsimd.memset(acc[:], 0.0)

        if _WARM:
            # touch Abs at t~0 so the ACT func table loads during the
            # initial DMA wait instead of on the first image's chain
            warm = cpool.tile([128, 1], BF16, name="warm", tag="warm")
            nc.gpsimd.memset(warm[:], 0.0)
            nc.scalar.activation(warm[:], warm[:], AF.Abs)

        if _PEWARM:
            # tiny dummy matmuls ramp the PE pstate during the initial DMA
            # wait so image 0's real matmuls run at full speed; dedicated
            # 1-bank PSUM pool so image tiles aren't delayed
            wsrc = cpool.tile([128, 192], BF16, name="wsrc", tag="wsrc")
            nc.gpsimd.memset(wsrc[:], 0.0)
            wpp = ctx.enter_context(tc.tile_pool(name="wpp", bufs=1,
                                                 space="PSUM"))
            wps = wpp.tile([128, 64], F32, name="wps", tag="wps")
            for n in range(_PEWARM):
                nc.tensor.matmul(out=wps[:], lhsT=wsrc[:, 0:128],
                                 rhs=wsrc[:, 128:192], start=True, stop=True)

        # pre-zero only the strip GAPS (interiors are overwritten each
        # image): head [0:31], the three inter-row gaps (strided view),
        # and the tail — per rotating buffer
        for b in range(_VBUFS):
            v1p0 = vpool.tile([128, _W1], BF16, name=f"v1p{b}", tag="v1p")
            nc.gpsimd.memset(v1p0[:, 0:_GAP], 0.0)
            gaps = v1p0[:, _STRIDE : _STRIDE + 3 * _STRIDE].rearrange(
                "p (j s) -> p j s", s=_STRIDE)[:, :, 0:_GAP]
            nc.gpsimd.memset(gaps, 0.0)
            nc.gpsimd.memset(v1p0[:, _GAP + 3 * _STRIDE + _W : _W1], 0.0)

        def front(i):
            """loads + H-filter matmuls + evac + W-scan for image i."""
            mb = ipool.tile([128, 4, _W], BF16, name="mb", tag="mb")
            nc.sync.dma_start(mb[:], mask_d[i].rearrange("(j p) w -> p j w", p=128))
            tbb = ipool.tile([128, 4, _W], BF16, name="tbb", tag="tbb")
            nc.sync.dma_start(tbb[:], tb_d[i].rearrange("(j p) w -> p j w", p=128))

            v1p = vpool.tile([128, _W1], BF16, name="v1p", tag="v1p")
            for k in range(2):  # row pairs (ih = 2k, 2k+1)
                v1ps = pup.tile([128, 2, _W], F32, name="v1ps", tag="v1ps")
                for ii in range(2):
                    ih = 2 * k + ii
                    js = [j for j in (ih - 1, ih, ih + 1) if 0 <= j < 4]
                    for n, j in enumerate(js):
                        nc.tensor.matmul(
                            out=v1ps[:, ii, :],
                            lhsT=band_sb[:, j, ts(ih, 128)],
                            rhs=mb[:, j, :],
                            start=(n == 0),
                            stop=(n == len(js) - 1),
                        )
                interior = v1p[
                    :, _GAP + 2 * k * _STRIDE : _GAP + (2 * k + 2) * _STRIDE
                ].rearrange("p (j w) -> p j w", w=_STRIDE)[:, :, 0:_W]
                if _EVAC_MAP[i] == "a":
                    nc.scalar.activation(interior, v1ps[:], AF.Copy,
                                         scale=1.0 / 961.0)
                else:
                    nc.vector.tensor_scalar(
                        out=interior, in0=v1ps[:], scalar1=1.0 / 961.0,
                        scalar2=None, op0=MALU.mult)

            sc = spool.tile([128, _L], BF16, name="sc", tag="sc")
            nc.vector.tensor_tensor_scan(
                out=sc[:, 0:_LS],
                data0=v1p[:, _GAP : _GAP + _LS],
                data1=v1p[:, 0:_LS],
                initial=0.0,
                op0=MALU.add,
                op1=MALU.subtract,
            )
            scv = sc[:, 15 : 15 + 4 * _STRIDE].rearrange(
                "p (j w) -> p j w", w=_STRIDE
            )[:, :, 0:_W]
            return mb, tbb, scv

        def back(i, mb, tbb, scv):
            """|sc - M| (+0.2) and the weighted sum for image i."""
            ad = apool.tile([128, 4, _W], BF16, name="ad", tag="ad")
            if i in _ABSD_IMGS:
                # fused d + abs + 0.2 + accum on DVE (1x custom);
                # acc[2i] = sum(|d| + 0.2)
                nc.vector._custom_dve(
                    ABSD, out=ad[:], in0=scv, in1=mb[:], s0=1.0, s1=0.2,
                    accum_out=acc[:, 2 * i : 2 * i + 1])
                w_bias = 0.0
            else:
                # d on Pool (TT sub), |d| + accum on ACT; acc[2i] = sum |d|
                d = dpool.tile([128, 4, _W], BF16, name="d", tag="d")
                deng = nc.vector if i in _D_DVE else nc.gpsimd
                deng.tensor_tensor(out=d[:], in0=scv, in1=mb[:],
                                   op=MALU.subtract)
                nc.scalar.activation(ad[:], d[:], AF.Abs,
                                     accum_out=acc[:, 2 * i : 2 * i + 1])
                w_bias = 0.2

            # w = (ad + w_bias) * tb, acc[2i+1] = sum(w)   (DVE custom)
            wout = wpool.tile([128, 4, _W], BF16, name="wout", tag="wout")
            nc.vector.affine_mul_reduce(
                out=wout[:], accum_out=acc[:, 2 * i + 1 : 2 * i + 2],
                in0=ad[:], in1=tbb[:], scale=1.0, bias=w_bias)

        # software pipeline: back(i) is emitted _SKEW images behind front
        # so no engine queue head-of-line blocks the next image's front
        inflight = {}
        for i in range(_BPC + _SKEW):
            if i < _BPC:
                inflight[i] = front(i)
            j = i - _SKEW
            if j >= 0:
                back(j, *inflight.pop(j))

        # -------- final 128-partition reduction (Pool, no PSUM) --------
        import concourse.bass_isa as bass_isa
        res = cpool.tile([128, 2 * _BPC], F32, name="res", tag="res")
        nc.gpsimd.partition_all_reduce(res[:], acc[:], channels=128,
                                       reduce_op=bass_isa.ReduceOp.add)
        nc.sync.dma_start(out_d[:], res[0:1, :])

    nc.compile()
    _CACHE["nc"] = nc
    return nc


def _prep_inputs(pred, mask):
    import ml_dtypes

    bf16 = ml_dtypes.bfloat16
    p = np.asarray(pred, np.float32).reshape(_B, _H, _W)
    m = np.asarray(mask, np.float32).reshape(_B, _H, _W)
    mb = np.ascontiguousarray(m.astype(bf16))
    # tb = sp - P*M - gu  (fp32 host math, one bf16 rounding at the end)
    E = np.exp(p)
    sp = np.log1p(E)
    gu = (E * (m + 1.0) + 1.0) / (2.0 * E + m + 1.0)
    tb = np.ascontiguousarray((sp - p * m - gu).astype(bf16))
    return mb, tb


def run_cores(pred, mask, trace=False, tmpdir=None):
    from concourse.bass_utils import run_bass_kernel_spmd

    nc = _build()
    mb, tb = _prep_inputs(pred, mask)
    band = _band_np()
    sl = lambda a, c: a[c * _BPC : (c + 1) * _BPC]
    in_maps = [
        {"mask": sl(mb, c), "tb": sl(tb, c), "band": band}
        for c in range(_NC)
    ]
    kw = {}
    if trace:
        kw = dict(trace=True, trace_cores=[0], tmpdir=tmpdir)
    br = run_bass_kernel_spmd(nc, in_maps, list(range(_NC)), **kw)
    outs = [br.results[c]["out"].reshape(2 * _BPC) for c in range(_NC)]
    return outs, br


def finish(outs):
    losses = []
    for c in range(_NC):
        o = outs[c].astype(np.float64)
        for i in range(_BPC):
            W = o[2 * i + 1]      # sum (|.| + 0.2) * tb
            if i in _ABSD_IMGS:
                sa = o[2 * i] - 0.2 * _HW   # acc held sum(|d| + 0.2)
            else:
                sa = o[2 * i]               # acc held sum |d|
            losses.append(5.0 * (W + sa + 0.2 * _HW) / (_HW + 5.0 * sa))
    return np.float32(np.mean(losses))


def kernel(pred, mask):
    outs, _ = run_cores(pred, mask)
    return finish(outs)


# revision 29
# speedup vs baseline: 1.1239x; 1.0644x over previous
"""Trainium2 Bass kernel: weighted BCE + IoU loss (structure loss).

Full inputs: pred/mask [64, 1, 512, 512] fp32.  Data-parallel over 8
NeuronCores (8 images per core).

Host-side prep (like the baseline's lmp/pm/rm1 channels) ships two
bf16 channels per image:
  M  = mask
  TB = tb = sp - P*M - gu        (pointwise; sp = log1p(e^P),
       gu = (e^P*(M+1)+1)/(2e^P+M+1);  bce+iou = tb + 1)

Device per image (the structural work):
  - H-direction 31-tap box filter: banded matmul on PE (f32 PSUM),
    evacuated with a 1/961 scale into a zero-gapped SBUF strip (ACT).
  - W-direction box filter: ONE tensor_tensor_scan with data1 a
    31-shifted view of the strip (running 31-window sum; zero gaps
    make edge handling automatic).
  - d  = sc - M                   (TensorTensor sub, 2x mode)
  - ad = |d|, acc A = sum(ad)     (TensorScalar abs_max, 4x mode)
  - W  = sum((ad + 0.2) * TB)     (scalar_tensor_tensor w/ accum)
  - final 128-partition reduce of [A_i; W_i] via ones-matmul on PE.

Host finish:  loss_i = 5*(W + A + 0.2*HW) / (HW + 5*A),  output mean.
"""

import os as _os
from contextlib import ExitStack

import numpy as np

_B = 64
_H = 512
_W = 512
_NC = 8
_BPC = _B // _NC
_HW = float(_H * _W)

_W1 = 2224   # gapped strip width
_L = 2192    # sc tile width
_LS = 2156   # scan op length
_GAP = 31
_STRIDE = 543  # 512 + 31

_CACHE = {}

# knobs (engine placement per image index)
def _imgset(env, default):
    return {int(x) for x in _os.environ.get(env, default).split(",") if x != ""}

_D_DVE = _imgset("K_D_DVE", "")            # d-sub on DVE instead of Pool
_ABSD_IMGS = _imgset("K_ABSD", "7")         # ABSD custom (DVE) instead of d+ACT-Abs
_EVAC = _os.environ.get("K_EVAC", "a,a,a,a,a,a,a,a")
_EVAC_MAP = (_EVAC.split(",") * 8)[:8] if "," in _EVAC else [_EVAC[0]] * 8
_IBUFS = int(_os.environ.get("K_IBUFS", "8"))
_VBUFS = int(_os.environ.get("K_VBUFS", "4"))
_PBUFS = int(_os.environ.get("K_PBUFS", "3"))
_SBUFS = int(_os.environ.get("K_SBUFS", "3"))
_DBUFS = int(_os.environ.get("K_DBUFS", "3"))
_SKEW = int(_os.environ.get("K_SKEW", "1"))
_HALF = _imgset("K_HALF", "0,1,2,3,4,5,6,7")
_WARM = _os.environ.get("K_WARM", "1") == "1"
_PEWARM = int(_os.environ.get("K_PEWARM", "6"))


def _band_np():
    import ml_dtypes

    idx = np.arange(_H)
    b = (np.abs(idx[:, None] - idx[None, :]) <= 15).astype(np.float32)
    return b.astype(ml_dtypes.bfloat16)


def _register_custom_ops():
    """Register the ABSD custom DVE op (idempotent):
    ap2 = |Src0*C0 - Src1| + C1, accum_out = sum(ap2)."""
    import concourse.dve_ops as dops
    from concourse.dve_spec import AluOp, Spec, Src0, Src1, Zero, lower, maxx
    from concourse.dve_uop import DveOpSpec
    from operator import add as _add

    if "ABSD_ANT" in dops.CUSTOM_DVE_SPECS:
        return

    from concourse.dve_spec import C0, C1

    dd = Src0 * C0 - Src1

    def _ref_absd(in0, in1, c0, c1, c2):
        b = (np.abs(in0.astype(np.float32) * c0 - in1) + c1).astype(np.float32)
        return b, b.reshape(b.shape[0], -1).sum(axis=-1, keepdims=True)

    absd = dops.DveOp(
        "ABSD_ANT",
        Spec(body=maxx(dd, Zero - dd) + C1, accum=_add, accum_init=Zero,
             reference=_ref_absd),
        subdim=False, uops_sha={},
    )

    uops = lower(absd.spec, ver="v3")
    sha = DveOpSpec(name=absd.name, opcode=0, uops=uops, rd1_en=True).sha("v3")
    pinned = dops.DveOp(absd.name, absd.spec, absd.subdim, {"v3": sha})

    base = max(dops._SUB_OPCODE_FOR_NAME.values())
    dops.OPS.append(pinned)
    dops.CUSTOM_DVE_SPECS[pinned.name] = pinned.spec
    dops._SUB_OPCODE_FOR_NAME[pinned.name] = base + 1
    assert max(dops._SUB_OPCODE_FOR_NAME.values()) < 0x20


def _build():
    if "nc" in _CACHE:
        return _CACHE["nc"]

    import concourse.bass as bass
    import concourse.tile as tile
    from concourse import bacc, mybir
    from concourse.alu_op_type import AluOpType as ALU

    AF = mybir.ActivationFunctionType
    MALU = mybir.AluOpType
    F32 = mybir.dt.float32
    BF16 = mybir.dt.bfloat16
    ts = bass.ts

    import concourse.dve_ops as dops

    _register_custom_ops()
    ABSD = next(o for o in dops.OPS if o.name == "ABSD_ANT")

    nc = bacc.Bacc("TRN2", target_bir_lowering=False, debug=False,
                   num_devices=_NC)

    mask_d = nc.dram_tensor("mask", [_BPC, _H, _W], BF16, kind="ExternalInput").ap()
    tb_d = nc.dram_tensor("tb", [_BPC, _H, _W], BF16, kind="ExternalInput").ap()
    band_d = nc.dram_tensor("band", [_H, _W], BF16, kind="ExternalInput").ap()
    out_d = nc.dram_tensor("out", [1, 4 * _BPC], F32, kind="ExternalOutput").ap()

    with tile.TileContext(nc) as tc, ExitStack() as ctx:
        cpool = ctx.enter_context(tc.tile_pool(name="cpool", bufs=1))
        ipool = ctx.enter_context(tc.tile_pool(name="ipool", bufs=_IBUFS))
        vpool = ctx.enter_context(tc.tile_pool(name="vpool", bufs=_VBUFS))
        spool = ctx.enter_context(tc.tile_pool(name="spool", bufs=_SBUFS))
        dpool = ctx.enter_context(tc.tile_pool(name="dpool", bufs=_DBUFS))
        apool = ctx.enter_context(tc.tile_pool(name="apool", bufs=_DBUFS))
        wpool = ctx.enter_context(tc.tile_pool(name="wpool", bufs=_DBUFS))
        pup = ctx.enter_context(tc.tile_pool(name="pup", bufs=_PBUFS, space="PSUM"))

        band_sb = cpool.tile([128, 4, _W], BF16, name="band_sb", tag="band_sb")
        nc.sync.dma_start(band_sb[:], band_d.rearrange("(j p) c -> p j c", p=128))
        acc = cpool.tile([128, 4 * _BPC], F32, name="acc", tag="acc")
        nc.gpsimd.memset(acc[:], 0.0)

        if _WARM:
            # touch Abs at t~0 so the ACT func table loads during the
            # initial DMA wait instead of on the first image's chain
            warm = cpool.tile([128, 1], BF16, name="warm", tag="warm")
            nc.gpsimd.memset(warm[:], 0.0)
            nc.scalar.activation(warm[:], warm[:], AF.Abs)

        if _PEWARM:
            # tiny dummy matmuls ramp the PE pstate during the initial DMA
            # wait so image 0's real matmuls run at full speed; dedicated
            # 1-bank PSUM pool so image tiles aren't delayed
            wsrc = cpool.tile([128, 192], BF16, name="wsrc", tag="wsrc")
            nc.gpsimd.memset(wsrc[:], 0.0)
            wpp = ctx.enter_context(tc.tile_pool(name="wpp", bufs=1,
                                                 space="PSUM"))
            wps = wpp.tile([128, 64], F32, name="wps", tag="wps")
            for n in range(_PEWARM):
                nc.tensor.matmul(out=wps[:], lhsT=wsrc[:, 0:128],
                                 rhs=wsrc[:, 128:192], start=True, stop=True)

        # pre-zero only the strip GAPS (interiors are overwritten each
        # image): head [0:31], the three inter-row gaps (strided view),
        # and the tail — per rotating buffer
        for b in range(_VBUFS):
            v1p0 = vpool.tile([128, _W1], BF16, name=f"v1p{b}", tag="v1p")
            nc.gpsimd.memset(v1p0[:, 0:_GAP], 0.0)
            gaps = v1p0[:, _STRIDE : _STRIDE + 3 * _STRIDE].rearrange(
                "p (j s) -> p j s", s=_STRIDE)[:, :, 0:_GAP]
            nc.gpsimd.memset(gaps, 0.0)
            nc.gpsimd.memset(v1p0[:, _GAP + 3 * _STRIDE + _W : _W1], 0.0)

        def front(i):
            """loads + H-filter matmuls + evac + W-scan for image i."""
            mb = ipool.tile([128, 4, _W], BF16, name="mb", tag="mb")
            nc.sync.dma_start(mb[:], mask_d[i].rearrange("(j p) w -> p j w", p=128))
            tbb = ipool.tile([128, 4, _W], BF16, name="tbb", tag="tbb")
            nc.sync.dma_start(tbb[:], tb_d[i].rearrange("(j p) w -> p j w", p=128))

            v1p = vpool.tile([128, _W1], BF16, name="v1p", tag="v1p")
            for k in range(2):  # row pairs (ih = 2k, 2k+1)
                v1ps = pup.tile([128, 2, _W], F32, name="v1ps", tag="v1ps")
                for ii in range(2):
                    ih = 2 * k + ii
                    js = [j for j in (ih - 1, ih, ih + 1) if 0 <= j < 4]
                    for n, j in enumerate(js):
                        nc.tensor.matmul(
                            out=v1ps[:, ii, :],
                            lhsT=band_sb[:, j, ts(ih, 128)],
                            rhs=mb[:, j, :],
                            start=(n == 0),
                            stop=(n == len(js) - 1),
                        )
                interior = v1p[
                    :, _GAP + 2 * k * _STRIDE : _GAP + (2 * k + 2) * _STRIDE
                ].rearrange("p (j w) -> p j w", w=_STRIDE)[:, :, 0:_W]
                if _EVAC_MAP[i] == "a":
                    nc.scalar.activation(interior, v1ps[:], AF.Copy,
                                         scale=1.0 / 961.0)
                else:
                    nc.vector.tensor_scalar(
                        out=interior, in0=v1ps[:], scalar1=1.0 / 961.0,
                        scalar2=None, op0=MALU.mult)

            sc = spool.tile([128, _L], BF16, name="sc", tag="sc")
            nc.vector.tensor_tensor_scan(
                out=sc[:, 0:_LS],
                data0=v1p[:, _GAP : _GAP + _LS],
                data1=v1p[:, 0:_LS],
                initial=0.0,
                op0=MALU.add,
                op1=MALU.subtract,
            )
            scv = sc[:, 15 : 15 + 4 * _STRIDE].rearrange(
                "p (j w) -> p j w", w=_STRIDE
            )[:, :, 0:_W]
            return mb, tbb, scv

        def back(i, mb, tbb, scv):
            """|sc - M| (+0.2) and the weighted sum for image i, split
            into row-pair halves to shorten the cross-engine chain."""
            halves = (0, 1) if i in _HALF else (slice(None),)
            ad = apool.tile([128, 4, _W], BF16, name="ad", tag="ad")
            wout = wpool.tile([128, 4, _W], BF16, name="wout", tag="wout")
            d = (None if i in _ABSD_IMGS else
                 dpool.tile([128, 4, _W], BF16, name="d", tag="d"))
            for h, hh in enumerate(halves):
                if hh == slice(None):
                    rows, a_col, w_col, nh = hh, 4 * i, 4 * i + 2, 1
                else:
                    rows = slice(2 * hh, 2 * hh + 2)
                    a_col, w_col, nh = 4 * i + hh, 4 * i + 2 + hh, 2
                scv_h, mb_h = scv[:, rows, :], mb[:, rows, :]
                if i in _ABSD_IMGS:
                    # fused d + abs + 0.2 + accum on DVE (1x custom);
                    # acc a_col sums (|d| + 0.2)
                    nc.vector._custom_dve(
                        ABSD, out=ad[:, rows, :], in0=scv_h, in1=mb_h,
                        s0=1.0, s1=0.2,
                        accum_out=acc[:, a_col : a_col + 1])
                    w_bias = 0.0
                else:
                    # d on Pool (TT sub), |d| + accum on ACT
                    deng = nc.vector if i in _D_DVE else nc.gpsimd
                    deng.tensor_tensor(out=d[:, rows, :], in0=scv_h,
                                       in1=mb_h, op=MALU.subtract)
                    nc.scalar.activation(ad[:, rows, :], d[:, rows, :],
                                         AF.Abs,
                                         accum_out=acc[:, a_col : a_col + 1])
                    w_bias = 0.2

                # w = (ad + w_bias) * tb, acc w_col = sum(w)   (DVE custom)
                nc.vector.affine_mul_reduce(
                    out=wout[:, rows, :],
                    accum_out=acc[:, w_col : w_col + 1],
                    in0=ad[:, rows, :], in1=tbb[:, rows, :],
                    scale=1.0, bias=w_bias)

        # software pipeline: back(i) is emitted _SKEW images behind front
        # so no engine queue head-of-line blocks the next image's front
        inflight = {}
        for i in range(_BPC + _SKEW):
            if i < _BPC:
                inflight[i] = front(i)
            j = i - _SKEW
            if j >= 0:
                back(j, *inflight.pop(j))

        # -------- final 128-partition reduction (Pool, no PSUM) --------
        import concourse.bass_isa as bass_isa
        res = cpool.tile([128, 4 * _BPC], F32, name="res", tag="res")
        nc.gpsimd.partition_all_reduce(res[:], acc[:], channels=128,
                                       reduce_op=bass_isa.ReduceOp.add)
        nc.sync.dma_start(out_d[:], res[0:1, :])

    nc.compile()
    _CACHE["nc"] = nc
    return nc


def _prep_inputs(pred, mask):
    import ml_dtypes

    bf16 = ml_dtypes.bfloat16
    p = np.asarray(pred, np.float32).reshape(_B, _H, _W)
    m = np.asarray(mask, np.float32).reshape(_B, _H, _W)
    mb = np.ascontiguousarray(m.astype(bf16))
    # tb = sp - P*M - gu  (fp32 host math, one bf16 rounding at the end)
    E = np.exp(p)
    sp = np.log1p(E)
    gu = (E * (m + 1.0) + 1.0) / (2.0 * E + m + 1.0)
    tb = np.ascontiguousarray((sp - p * m - gu).astype(bf16))
    return mb, tb


def run_cores(pred, mask, trace=False, tmpdir=None):
    from concourse.bass_utils import run_bass_kernel_spmd

    nc = _build()
    mb, tb = _prep_inputs(pred, mask)
    band = _band_np()
    sl = lambda a, c: a[c * _BPC : (c + 1) * _BPC]
    in_maps = [
        {"mask": sl(mb, c), "tb": sl(tb, c), "band": band}
        for c in range(_NC)
    ]
    kw = {}
    if trace:
        kw = dict(trace=True, trace_cores=[0], tmpdir=tmpdir)
    br = run_bass_kernel_spmd(nc, in_maps, list(range(_NC)), **kw)
    outs = [br.results[c]["out"].reshape(4 * _BPC) for c in range(_NC)]
    return outs, br


def finish(outs):
    losses = []
    for c in range(_NC):
        o = outs[c].astype(np.float64)
        for i in range(_BPC):
            A = o[4 * i] + o[4 * i + 1]
            W = o[4 * i + 2] + o[4 * i + 3]
            # ABSD images accumulated sum(|d| + 0.2) (in halves or not):
            # total is sum|d| + 0.2*HW either way
            sa = A - 0.2 * _HW if i in _ABSD_IMGS else A
            losses.append(5.0 * (W + sa + 0.2 * _HW) / (_HW + 5.0 * sa))
    return np.float32(np.mean(losses))


def kernel(pred, mask):
    outs, _ = run_cores(pred, mask)
    return finish(outs)
